# revision 18
# baseline (speedup 1.0000x reference)
"""Bahdanau-attention LSTM decoder on 8 Trainium2 NeuronCores.

Sharding: attention by batch (16 rows/core), LSTM gate dim 8-way (512 gate
dims/core -> h-slice of 128 dims/core), per-step AllGather of h.T and ctx.T.
All 16-bit tensors fp16; accumulations and c-state fp32.

Self-contained: hardcodes V,H,B,S,T = 512,1024,128,256,128.
"""
import sys
import types
import numpy as np

# ---- NTFF profile hook shim (antenv.axon_hooks missing in this image) ----
_hook = [None]


def _install_harness():
    try:
        import antenv
        if "antenv.axon_hooks" not in sys.modules:
            mod = types.ModuleType("antenv.axon_hooks")
            mod.set_axon_ntff_profile_hook = lambda h: _hook.__setitem__(0, h)
            mod.get_axon_ntff_profile_hook = lambda: _hook[0]
            sys.modules["antenv.axon_hooks"] = mod
            antenv.axon_hooks = mod
        from trn_agent_boot.trn_boot import _ntff_profile_via_ctypes
        _hook[0] = _ntff_profile_via_ctypes("/opt/axon/libaxon_pjrt.so")
        import concourse.bass_utils as bass_utils
        bass_utils.upload_artifacts = lambda tmpdir: tmpdir
    except Exception:
        pass


_install_harness()

import concourse.bass as bass
import concourse.bacc as bacc
import concourse.mybir as mybir
import concourse.tile as tile
from concourse.bass_utils import run_bass_kernel_spmd

NC = 8
V, H, B, S = 512, 1024, 128, 256
BL = B // NC          # 16 local batch rows
GL = 4 * H // NC      # 512 local gate dims
HD = H // NC          # 128 local h dims
DT = mybir.dt
F32, F16 = DT.float32, DT.float16
AF = mybir.ActivationFunctionType
ALU = mybir.AluOpType
RG = [list(range(NC))]

_cache = {}


def build(T):
    nc = bacc.Bacc("TRN2", target_bir_lowering=False, debug=False,
                   num_devices=NC)

    def inp(name, shape, dt=F16):
        return nc.dram_tensor(name, shape, dt, kind="ExternalInput")

    enc_in = inp("enc_sb", [128, BL, 2, H])            # [s%128, b, s//128, h]
    encT_in = inp("encT_sb", [128, 8, BL, S])          # [h%128, h//128, b, s]
    UaT_in = inp("UaT", [128, 8, 8, 128])
    kbias_in = inp("kbias", [128, 8], F32)
    WaT_in = inp("WaT", [128, 8, 8, 128])
    VaM_in = inp("VaM", [128, 8, 16])
    Wemb_in = inp("W_embT", [128, 8, GL])
    Wctx_in = inp("W_ctxT", [128, 8, GL])
    Whh_in = inp("W_hhT", [128, 8, GL])
    brow_in = inp("bias_row", [1, GL])
    ones_in = inp("ones128", [1, 128])
    id8_in = inp("ident8", [8, 8])
    id128_in = inp("ident128", [128, 128])
    h0_in = inp("h0T", [128, 8, 128])
    c0_in = inp("c0", [128, HD], F32)
    xs_in = inp("xsT", [T, 128, 8, 128])
    outw_in = inp("out_wT", [128, 8, V])
    outb_in = inp("outb_row", [1, V])

    logp_out = nc.dram_tensor("logp", [T // 8, 128, V], F32,
                              kind="ExternalOutput")
    attn_out = nc.dram_tensor("attnw", [T, BL, S], F32, kind="ExternalOutput")
    hfin_out = nc.dram_tensor("hfin", [128, 8, 128], F16,
                              kind="ExternalOutput")
    hT_store = nc.dram_tensor("hT_store", [T, 128, 8, BL], F16)

    with tile.TileContext(nc) as tc:
        with (
            tc.tile_pool(name="perm", bufs=1) as perm,
            tc.tile_pool(name="wb", bufs=2) as wb,
            tc.tile_pool(name="sc", bufs=3) as sc,
            tc.tile_pool(name="sm", bufs=2) as sm,
            tc.tile_pool(name="sef", bufs=1) as sef,
            tc.tile_pool(name="pe4", bufs=4, space="PSUM") as pe4,
            tc.tile_pool(name="pg", bufs=2, space="PSUM") as pg,
            tc.tile_pool(name="pc", bufs=1, space="PSUM") as pc,
            tc.tile_pool(name="pt", bufs=1, space="PSUM") as pt,
            tc.tile_pool(name="dram", bufs=2, space="DRAM") as dram,
        ):
            pid = nc.vector.partition_id()

            # ---------- resident loads ----------
            big_raw = perm.tile([128, 32768], F16, tag="big")  # enc slab (two layouts)
            keys = perm.tile([128, 8, BL, S], F16, tag="keys")
            wa = perm.tile([128, 8, 8, 128], F16, tag="wa")    # UaT then WaT
            kbias = perm.tile([128, 8], F32, tag="kbias")
            vam = perm.tile([128, 8, 16], F16, tag="vam")
            wemb = perm.tile([128, 8, GL], F16, tag="wemb")
            wctx = perm.tile([128, 8, GL], F16, tag="wctx")
            whh = perm.tile([128, 8, GL], F16, tag="whh")
            brow = perm.tile([1, GL], F16, tag="brow")
            ones = perm.tile([1, 128], F16, tag="ones")
            id8 = perm.tile([8, 8], F16, tag="id8")
            id128 = perm.tile([128, 128], F16, tag="id128")
            c_sb = perm.tile([128, HD], F32, tag="c")

            nc.sync.dma_start(kbias[:], kbias_in[:])
            nc.sync.dma_start(vam[:], VaM_in[:])
            nc.sync.dma_start(wemb[:], Wemb_in[:])
            nc.sync.dma_start(wctx[:], Wctx_in[:])
            nc.sync.dma_start(whh[:], Whh_in[:])
            nc.sync.dma_start(brow[:], brow_in[:])
            nc.sync.dma_start(ones[:], ones_in[:])
            nc.sync.dma_start(id8[:], id8_in[:])
            nc.sync.dma_start(id128[:], id128_in[:])
            nc.sync.dma_start(c_sb[:], c0_in[:])

            # ---------- phase 0: keys = enc @ Ua.T + (Ua_b + Wa_b) ----------
            # encT (h-major) shares SBUF with nothing big yet; 'big' loads after.
            nc.sync.dma_start(wa[:], UaT_in[:])
            encT = big_raw[:].rearrange("p (k b s) -> p k b s", k=8, b=BL)
            nc.sync.dma_start(encT, encT_in[:])
            for b2 in range(BL // 2):     # pairs of b -> N=512
                for hcp in range(8):
                    ps_k = pe4.tile([128, 512], F32, tag="e")
                    for kc in range(8):
                        nc.tensor.matmul(
                            ps_k[:], wa[:, kc, hcp, :],
                            encT[:, kc, 2 * b2:2 * b2 + 2, :],
                            start=(kc == 0), stop=(kc == 7))
                    for bb in range(2):
                        nc.vector.tensor_scalar_add(
                            keys[:, hcp, 2 * b2 + bb, :],
                            ps_k[:, 256 * bb:256 * bb + 256],
                            kbias[:, hcp:hcp + 1])
            # reuse the slab: overwrite with ctx-layout enc; reuse wa for WaT
            big = big_raw[:].rearrange("p (b sh h) -> p b sh h", b=BL, sh=2)
            nc.sync.dma_start(big, enc_in[:])
            nc.sync.dma_start(wa[:], WaT_in[:])

            hT_g = []
            for i in range(2):
                hTg_i = perm.tile([128, 8, 128], F16, tag=f"hTg{i}")
                hT_g.append(hTg_i)
            nc.sync.dma_start(hT_g[0][:], h0_in[:])
            hTb_prev = sm.tile([128, 8, BL], F16, tag="hTb")
            nc.vector.tensor_copy(hTb_prev[:],
                                  hT_g[0][:, :, bass.ts(pid, BL)])

            sig_if = perm.tile([128, 256], F32, tag="sigif")
            tg = perm.tile([128, HD], F32, tag="tg")
            so = perm.tile([128, HD], F32, tag="so")
            tmp1 = perm.tile([128, HD], F32, tag="tmp1")
            tmp2 = perm.tile([128, HD], F32, tag="tmp2")
            tc_f32 = perm.tile([128, HD], F32, tag="tcf")
            h16 = perm.tile([128, HD], F16, tag="h16")

            for t in range(T):
                pp = t % 2
                xs_sb = wb.tile([128, 8, 128], F16, tag="xs")
                nc.sync.dma_start(xs_sb[:], xs_in[t])

                # ---- q.T [128, 8hc*16] ----
                ps_q = pt.tile([128, 128], F32, tag="tp")
                for hc in range(8):
                    for kc in range(8):
                        nc.tensor.matmul(
                            ps_q[:, 16 * hc:16 * hc + 16],
                            wa[:, kc, hc, :],
                            hTb_prev[:, kc, :],
                            start=(kc == 0), stop=(kc == 7))
                qT = sm.tile([128, 128], F32, tag="qT")
                nc.vector.tensor_copy(qT[:], ps_q[:])

                half_ag = []
                for half in range(2):
                    ctxT = sm.tile([128, 8, 8], F16, tag="ctxT")
                    ps_e = []
                    for _pi in range(4):
                        ps_e_i = pe4.tile([16, 512], F32, tag="e")
                        ps_e.append(ps_e_i)
                    for hc in range(8):
                        scr = sc.tile([128, 8, 256], F16, tag="scr")
                        for bb in range(8):
                            b = 8 * half + bb
                            nc.vector.tensor_scalar_add(
                                scr[:, bb, :], keys[:, hc, b, :],
                                qT[:, 16 * hc + b:16 * hc + b + 1])
                        nc.scalar.activation(scr[:], scr[:], AF.Tanh)
                        for i in range(4):
                            nc.tensor.matmul(
                                ps_e[i][:], vam[:, hc, :],
                                scr[:].rearrange("p e s -> p (e s)")[
                                    :, 512 * i:512 * i + 512],
                                start=(hc == 0), stop=(hc == 7))
                    e_sb = sm.tile([8, 256], F32, tag="esb")
                    e_flat = sef.tile([128, 512], F32, tag="e512")
                    for i in range(4):
                        nc.vector.tensor_copy(
                            e_flat[32 * i:32 * i + 1, :], ps_e[i][0:1, :])
                    nc.sync.dma_start(
                        e_sb[:],
                        e_flat[0:128:32, :].rearrange("q (b s) -> q b s", b=2))
                    w_sb = sm.tile([8, 256], F32, tag="wsb")
                    sum_e = sm.tile([8, 1], F32, tag="sume")
                    nc.scalar.activation(w_sb[:], e_sb[:], AF.Exp,
                                         accum_out=sum_e[:])
                    rcp = sm.tile([8, 1], F32, tag="rcp")
                    nc.vector.reciprocal(rcp[:], sum_e[:])
                    nc.vector.tensor_scalar_mul(w_sb[:], w_sb[:], rcp[:])
                    nc.sync.dma_start(
                        attn_out[t, 8 * half:8 * half + 8, :], w_sb[:])
                    w16 = sm.tile([8, 256], F16, tag="w16")
                    nc.vector.tensor_copy(w16[:], w_sb[:])
                    wT = sm.tile([128, 2, 8], F16, tag="wT")
                    for sh in range(2):
                        ps_w = pt.tile([128, 8], F16, tag="tp")
                        nc.tensor.transpose(
                            ps_w[:], w16[:, 128 * sh:128 * sh + 128], id8[:])
                        nc.vector.tensor_copy(wT[:, sh, :], ps_w[:])
                    # ctx: per b, per h-chunk: [128s,128h].T @ w[b] col
                    for hc in range(8):
                        ps_c8 = pc.tile([128, 8], F32, tag="ctx")
                        for bb in range(8):
                            b = 8 * half + bb
                            for sh in range(2):
                                nc.tensor.matmul(
                                    ps_c8[:, bb:bb + 1],
                                    big[:, b, sh, 128 * hc:128 * hc + 128],
                                    wT[:, sh, bb:bb + 1],
                                    start=(sh == 0), stop=(sh == 1))
                        nc.vector.tensor_copy(ctxT[:, hc, :], ps_c8[:])
                    cin = dram.tile([1024, 8], F16, tag=f"cin{half}")
                    cout = dram.tile([8192, 8], F16, tag=f"cout{half}")
                    nc.sync.dma_start(
                        cin[:].rearrange("(k p) b -> p k b", p=128), ctxT[:])
                    nc.gpsimd.collective_compute(
                        "AllGather", ALU.bypass, replica_groups=RG,
                        ins=[cin[:].opt()], outs=[cout[:].opt()])
                    half_ag.append(cout)
                ctxg = wb.tile([128, 8, 8, 2, 8], F16, tag="ctxg")
                for half in range(2):
                    cview = half_ag[half][:].rearrange(
                        "(r k p) b -> k p r b", r=8, p=128)
                    for kc in range(8):
                        nc.gpsimd.dma_start(
                            ctxg[:, kc, :, half, :], cview[kc])

                # ---- gates ----
                ps_g = pg.tile([128, GL], F32, tag="g")
                nc.tensor.matmul(ps_g[:], ones[:], brow[:], start=True,
                                 stop=False)
                for kc in range(8):
                    nc.tensor.matmul(ps_g[:], xs_sb[:, kc, :],
                                     wemb[:, kc, :], start=False, stop=False)
                for kc in range(8):
                    nc.tensor.matmul(ps_g[:], hT_g[pp][:, kc, :],
                                     whh[:, kc, :], start=False, stop=False)
                for kc in range(8):
                    nc.tensor.matmul(
                        ps_g[:],
                        ctxg[:, kc, :, :, :].rearrange("p r h b -> p (r h b)"),
                        wctx[:, kc, :], start=False, stop=(kc == 7))
                # ---- pointwise LSTM ----
                nc.scalar.activation(sig_if[:], ps_g[:, 0:256], AF.Tanh,
                                     scale=0.5)
                nc.vector.tensor_scalar(sig_if[:], sig_if[:], 0.5, 0.5,
                                        ALU.mult, ALU.add)
                nc.scalar.activation(tg[:], ps_g[:, 256:384], AF.Tanh)
                nc.scalar.activation(so[:], ps_g[:, 384:512], AF.Tanh,
                                     scale=0.5)
                nc.vector.tensor_scalar(so[:], so[:], 0.5, 0.5,
                                        ALU.mult, ALU.add)
                nc.vector.tensor_tensor(tmp1[:], sig_if[:, 128:256], c_sb[:],
                                        ALU.mult)
                nc.vector.tensor_tensor(tmp2[:], sig_if[:, 0:128], tg[:],
                                        ALU.mult)
                nc.vector.tensor_tensor(c_sb[:], tmp1[:], tmp2[:], ALU.add)
                nc.scalar.activation(tc_f32[:], c_sb[:], AF.Tanh)
                nc.vector.tensor_tensor(h16[:], so[:], tc_f32[:], ALU.mult)

                # ---- h.T slice + AllGather h ----
                ps_h = pt.tile([128, 128], F16, tag="tp")
                nc.tensor.transpose(ps_h[:], h16[:], id128[:])
                hT_own = sm.tile([128, 128], F16, tag="hTown")
                nc.vector.tensor_copy(hT_own[:], ps_h[:])
                hin = dram.tile([128, 128], F16, tag="hin")
                hout = dram.tile([1024, 128], F16, tag="hout")
                nc.sync.dma_start(hin[:], hT_own[:])
                nc.gpsimd.collective_compute(
                    "AllGather", ALU.bypass, replica_groups=RG,
                    ins=[hin[:].opt()], outs=[hout[:].opt()])
                nxt = hT_g[(t + 1) % 2]
                nc.sync.dma_start(
                    nxt[:], hout[:].rearrange("(k p) b -> p k b", p=128))
                # own-b h.T for tail logits (and next step's q input)
                hTb = sm.tile([128, 8, BL], F16, tag="hTb")
                nc.vector.tensor_copy(hTb[:], nxt[:, :, bass.ts(pid, BL)])
                nc.sync.dma_start(hT_store[t], hTb[:])
                hTb_prev = hTb

            # final h (gathered, f16; host casts)
            nc.sync.dma_start(hfin_out[:], hT_g[T % 2][:])

            # ---------- tail: logits + log_softmax ----------
            outw = perm.tile([128, 8, V], F16, tag="wctx")
            outb = perm.tile([1, V], F16, tag="brow")
            nc.sync.dma_start(outw[:], outw_in[:])
            nc.sync.dma_start(outb[:], outb_in[:])
            for tb in range(T // 8):
                lsT = wb.tile([128, 8, 8, BL], F16, tag="xs")
                for tt in range(8):
                    nc.sync.dma_start(lsT[:, :, tt, :], hT_store[8 * tb + tt])
                ps_l = pg.tile([128, V], F32, tag="g")
                nc.tensor.matmul(ps_l[:], ones[:], outb[:], start=True,
                                 stop=False)
                for kc in range(8):
                    nc.tensor.matmul(
                        ps_l[:],
                        lsT[:, kc, :, :].rearrange("p t b -> p (t b)"),
                        outw[:, kc, :], start=False, stop=(kc == 7))
                nmax = sm.tile([128, 1], F32, tag="nmax")
                nc.vector.tensor_reduce(nmax[:], ps_l[:], mybir.AxisListType.X,
                                        ALU.max, negate=True)
                ex = sc.tile([128, V], F32, tag="scr")
                sume = sm.tile([128, 1], F32, tag="sume2")
                nc.scalar.activation(ex[:], ps_l[:], AF.Exp, bias=nmax[:],
                                     accum_out=sume[:])
                lz = sm.tile([128, 1], F32, tag="lz")
                nc.scalar.activation(lz[:], sume[:], getattr(AF, 'Ln', getattr(AF, 'Log', None)))
                nc.vector.tensor_tensor(lz[:], lz[:], nmax[:], ALU.subtract)
                lp = sc.tile([128, V], F32, tag="scr")
                nc.vector.tensor_scalar(lp[:], ps_l[:], lz[:], None,
                                        ALU.subtract)
                nc.sync.dma_start(logp_out[tb], lp[:])
    nc.compile()
    return nc


def _prep_inputs(encoder_outputs, encoder_hidden, encoder_cell, target_tensor,
                 max_length, emb, Wa_w, Wa_b, Ua_w, Ua_b, Va_w, Va_b,
                 W_ih, W_hh, b_ih, b_hh, out_w, out_b, T):
    f16 = np.float16
    tok = np.concatenate([np.zeros((B, 1), target_tensor.dtype),
                          target_tensor[:, :T - 1]], axis=1)  # [B, T]
    xs = emb[tok]                                   # [B, T, H] f32
    in_maps = []
    for j in range(NC):
        bl = slice(BL * j, BL * (j + 1))
        encl = encoder_outputs[bl]                  # [16, 256, 1024]
        gsel = np.concatenate([np.arange(128 * j + g * H,
                                         128 * j + g * H + 128)
                               for g in range(4)])  # own gate dims
        m = {}
        m["enc_sb"] = np.ascontiguousarray(
            encl.reshape(BL, 2, 128, H).transpose(2, 0, 1, 3)).astype(f16)
        m["encT_sb"] = np.ascontiguousarray(
            encl.transpose(2, 0, 1).reshape(8, 128, BL, S)
            .transpose(1, 0, 2, 3)).astype(f16)
        m["UaT"] = np.ascontiguousarray(
            Ua_w.T.reshape(8, 128, 8, 128).transpose(1, 0, 2, 3)).astype(f16)
        m["kbias"] = np.ascontiguousarray(
            (Ua_b + Wa_b).reshape(8, 128).T).astype(np.float32)
        m["WaT"] = np.ascontiguousarray(
            Wa_w.T.reshape(8, 128, 8, 128).transpose(1, 0, 2, 3)).astype(f16)
        m["VaM"] = np.ascontiguousarray(
            np.broadcast_to(Va_w[0].reshape(8, 128).T[:, :, None],
                            (128, 8, 16))).astype(f16)
        m["W_embT"] = np.ascontiguousarray(
            W_ih[gsel, :H].T.reshape(8, 128, GL)
            .transpose(1, 0, 2)).astype(f16)
        m["W_ctxT"] = np.ascontiguousarray(
            W_ih[gsel, H:].T.reshape(8, 128, GL)
            .transpose(1, 0, 2)).astype(f16)
        m["W_hhT"] = np.ascontiguousarray(
            W_hh[gsel].T.reshape(8, 128, GL).transpose(1, 0, 2)).astype(f16)
        m["bias_row"] = (b_ih + b_hh)[gsel][None, :].astype(f16)
        m["ones128"] = np.ones((1, 128), f16)
        m["ident8"] = np.eye(8, dtype=f16)
        m["ident128"] = np.eye(128, dtype=f16)
        m["h0T"] = np.ascontiguousarray(
            encoder_hidden[0].T.reshape(8, 128, 128)
            .transpose(1, 0, 2)).astype(f16)
        m["c0"] = encoder_cell[0][:, 128 * j:128 * j + 128].astype(np.float32)
        m["xsT"] = np.ascontiguousarray(
            xs[:, :T].transpose(1, 2, 0).reshape(T, 8, 128, B)
            .transpose(0, 2, 1, 3)).astype(f16)
        m["out_wT"] = np.ascontiguousarray(
            out_w.T.reshape(8, 128, V).transpose(1, 0, 2)).astype(f16)
        m["outb_row"] = out_b[None, :].astype(f16)
        in_maps.append(m)
    return in_maps


def T_pad(T):
    return T


def kernel(encoder_outputs, encoder_hidden, encoder_cell, target_tensor,
           max_length, emb, Wa_w, Wa_b, Ua_w, Ua_b, Va_w, Va_b,
           W_ih, W_hh, b_ih, b_hh, out_w, out_b, _trace=False, _T=None):
    args = [np.asarray(a) for a in
            (encoder_outputs, encoder_hidden, encoder_cell, target_tensor,
             max_length, emb, Wa_w, Wa_b, Ua_w, Ua_b, Va_w, Va_b,
             W_ih, W_hh, b_ih, b_hh, out_w, out_b)]
    T = int(_T if _T is not None else args[4])
    if "nc" not in _cache or _cache.get("T") != T:
        _cache["nc"] = build(T)
        _cache["T"] = T
    in_maps = _prep_inputs(*args, T=T)
    res = run_bass_kernel_spmd(_cache["nc"], in_maps, list(range(NC)),
                               trace=_trace)
    if _trace:
        _cache["last"] = res
    # assemble outputs
    logp = np.zeros((B, T, V), np.float32)
    attn = np.zeros((B, T, S), np.float32)
    for j in range(NC):
        r = res.results[j]
        lp = r["logp"].reshape(T // 8, 8, BL, V).transpose(2, 0, 1, 3) \
            .reshape(BL, T, V)
        logp[BL * j:BL * (j + 1)] = lp
        attn[BL * j:BL * (j + 1)] = r["attnw"].transpose(1, 0, 2)
    hf = res.results[0]["hfin"]                     # [128 p, 8 kc, 128 b]
    h_fin = hf.transpose(2, 1, 0).reshape(B, H)     # [b, h]
    return logp, h_fin[None].astype(np.float32), attn


if __name__ == "__main__":
    pass


# revision 20
# speedup vs baseline: 14.9482x; 14.9482x over previous
"""Bahdanau-attention LSTM decoder on 8 Trainium2 NeuronCores.

Sharding: attention by batch (16 rows/core), LSTM gate dim 8-way (512 gate
dims/core -> h-slice of 128 dims/core), per-step AllGather of h.T and ctx.T.
All 16-bit tensors fp16; accumulations and c-state fp32.

Self-contained: hardcodes V,H,B,S,T = 512,1024,128,256,128.
"""
import sys
import types
import numpy as np

# ---- NTFF profile hook shim (antenv.axon_hooks missing in this image) ----
_hook = [None]


def _install_harness():
    try:
        import antenv
        if "antenv.axon_hooks" not in sys.modules:
            mod = types.ModuleType("antenv.axon_hooks")
            mod.set_axon_ntff_profile_hook = lambda h: _hook.__setitem__(0, h)
            mod.get_axon_ntff_profile_hook = lambda: _hook[0]
            sys.modules["antenv.axon_hooks"] = mod
            antenv.axon_hooks = mod
        from trn_agent_boot.trn_boot import _ntff_profile_via_ctypes
        _hook[0] = _ntff_profile_via_ctypes("/opt/axon/libaxon_pjrt.so")
        import concourse.bass_utils as bass_utils
        bass_utils.upload_artifacts = lambda tmpdir: tmpdir
    except Exception:
        pass


_install_harness()

import concourse.bass as bass
import concourse.bacc as bacc
import concourse.mybir as mybir
import concourse.tile as tile
from concourse.bass_utils import run_bass_kernel_spmd

NC = 8
V, H, B, S = 512, 1024, 128, 256
BL = B // NC          # 16 local batch rows
GL = 4 * H // NC      # 512 local gate dims
HD = H // NC          # 128 local h dims
DT = mybir.dt
F32, F16 = DT.float32, DT.float16
AF = mybir.ActivationFunctionType
ALU = mybir.AluOpType
RG = [list(range(NC))]

_cache = {}


def build(T):
    nc = bacc.Bacc("TRN2", target_bir_lowering=False, debug=False,
                   num_devices=NC)

    def inp(name, shape, dt=F16):
        return nc.dram_tensor(name, shape, dt, kind="ExternalInput")

    enc_in = inp("enc_sb", [128, BL, 2, H])            # [s%128, b, s//128, h]
    encT_in = inp("encT_sb", [128, 8, BL, S])          # [h%128, h//128, b, s]
    UaT_in = inp("UaT", [128, 8, 8, 128])
    kbias_in = inp("kbias", [128, 8], F32)
    WaT_in = inp("WaT", [128, 8, 8, 128])
    VaM_in = inp("VaM", [128, 8, 16])
    Wemb_in = inp("W_embT", [128, 8, GL])
    Wctx_in = inp("W_ctxT", [128, 8, GL])
    Whh_in = inp("W_hhT", [128, 8, GL])
    brow_in = inp("bias_row", [1, GL])
    ones_in = inp("ones128", [1, 128])
    id8_in = inp("ident8", [8, 8])
    id128_in = inp("ident128", [128, 128])
    h0_in = inp("h0T", [128, 8, 128])
    c0_in = inp("c0", [128, HD], F32)
    xs_in = inp("xsT", [T, 128, 8, 128])
    outw_in = inp("out_wT", [128, 8, V])
    outb_in = inp("outb_row", [1, V])

    logp_out = nc.dram_tensor("logp", [T // 8, 128, V], F32,
                              kind="ExternalOutput")
    attn_out = nc.dram_tensor("attnw", [T, BL, S], F32, kind="ExternalOutput")
    hfin_out = nc.dram_tensor("hfin", [128, 8, 128], F16,
                              kind="ExternalOutput")
    hT_store = nc.dram_tensor("hT_store", [T, 128, 8, BL], F16)

    with tile.TileContext(nc) as tc:
        with (
            tc.tile_pool(name="perm", bufs=1) as perm,
            tc.tile_pool(name="wb", bufs=2) as wb,
            tc.tile_pool(name="wc1", bufs=1) as wc1,
            tc.tile_pool(name="sc", bufs=3) as sc,
            tc.tile_pool(name="sm", bufs=2) as sm,
            tc.tile_pool(name="sef", bufs=1) as sef,
            tc.tile_pool(name="pe4", bufs=4, space="PSUM") as pe4,
            tc.tile_pool(name="pg", bufs=2, space="PSUM") as pg,
            tc.tile_pool(name="pc", bufs=1, space="PSUM") as pc,
            tc.tile_pool(name="pt", bufs=1, space="PSUM") as pt,
            tc.tile_pool(name="dram", bufs=2, space="DRAM") as dram,
        ):
            pid = nc.vector.partition_id()

            # ---------- resident loads ----------
            big_raw = perm.tile([128, 32768], F16, tag="big")  # enc slab (two layouts)
            keys = perm.tile([128, 8, BL, S], F16, tag="keys")
            wa = perm.tile([128, 8, 8, 128], F16, tag="wa")    # UaT then WaT
            kbias = perm.tile([128, 8], F32, tag="kbias")
            vam = perm.tile([128, 8, 16], F16, tag="vam")
            wemb = perm.tile([128, 8, GL], F16, tag="wemb")
            wctx = perm.tile([128, 8, GL], F16, tag="wctx")
            whh = perm.tile([128, 8, GL], F16, tag="whh")
            brow = perm.tile([1, GL], F16, tag="brow")
            ones = perm.tile([1, 128], F16, tag="ones")
            id8 = perm.tile([8, 8], F16, tag="id8")
            id128 = perm.tile([128, 128], F16, tag="id128")
            c_sb = perm.tile([128, HD], F32, tag="c")

            nc.sync.dma_start(kbias[:], kbias_in[:])
            nc.sync.dma_start(vam[:], VaM_in[:])
            nc.sync.dma_start(wemb[:], Wemb_in[:])
            nc.sync.dma_start(wctx[:], Wctx_in[:])
            nc.sync.dma_start(whh[:], Whh_in[:])
            nc.sync.dma_start(brow[:], brow_in[:])
            nc.sync.dma_start(ones[:], ones_in[:])
            nc.sync.dma_start(id8[:], id8_in[:])
            nc.sync.dma_start(id128[:], id128_in[:])
            nc.sync.dma_start(c_sb[:], c0_in[:])

            # ---------- phase 0: keys = enc @ Ua.T + (Ua_b + Wa_b) ----------
            # encT (h-major) shares SBUF with nothing big yet; 'big' loads after.
            nc.sync.dma_start(wa[:], UaT_in[:])
            encT = big_raw[:].rearrange("p (k b s) -> p k b s", k=8, b=BL)
            nc.sync.dma_start(encT, encT_in[:])
            for b2 in range(BL // 2):     # pairs of b -> N=512
                for hcp in range(8):
                    ps_k = pe4.tile([128, 512], F32, tag="e")
                    for kc in range(8):
                        nc.tensor.matmul(
                            ps_k[:], wa[:, kc, hcp, :],
                            encT[:, kc, 2 * b2:2 * b2 + 2, :],
                            start=(kc == 0), stop=(kc == 7))
                    for bb in range(2):
                        nc.vector.tensor_scalar_add(
                            keys[:, hcp, 2 * b2 + bb, :],
                            ps_k[:, 256 * bb:256 * bb + 256],
                            kbias[:, hcp:hcp + 1])
            # reuse the slab: overwrite with ctx-layout enc; reuse wa for WaT
            big = big_raw[:].rearrange("p (b sh h) -> p b sh h", b=BL, sh=2)
            nc.sync.dma_start(big, enc_in[:])
            nc.sync.dma_start(wa[:], WaT_in[:])

            hT_g = []
            for i in range(2):
                hTg_i = perm.tile([128, 8, 128], F16, tag=f"hTg{i}")
                hT_g.append(hTg_i)
            nc.sync.dma_start(hT_g[0][:], h0_in[:])
            hTb_prev = sm.tile([128, 8, BL], F16, tag="hTb")
            nc.vector.tensor_copy(hTb_prev[:],
                                  hT_g[0][:, :, bass.ts(pid, BL)])

            sig_if = perm.tile([128, 256], F32, tag="sigif")
            tg = perm.tile([128, HD], F32, tag="tg")
            so = perm.tile([128, HD], F32, tag="so")
            tmp1 = perm.tile([128, HD], F32, tag="tmp1")
            tmp2 = perm.tile([128, HD], F32, tag="tmp2")
            tc_f32 = perm.tile([128, HD], F32, tag="tcf")
            h16 = perm.tile([128, HD], F16, tag="h16")

            for t in range(T):
                pp = t % 2
                xs_sb = wb.tile([128, 8, 128], F16, tag="xs")
                nc.sync.dma_start(xs_sb[:], xs_in[t])

                # ---- q.T [128, 8hc*16] ----
                ps_q = pt.tile([128, 128], F32, tag="tp")
                for hc in range(8):
                    for kc in range(8):
                        nc.tensor.matmul(
                            ps_q[:, 16 * hc:16 * hc + 16],
                            wa[:, kc, hc, :],
                            hTb_prev[:, kc, :],
                            start=(kc == 0), stop=(kc == 7))
                qT = sm.tile([128, 128], F32, tag="qT")
                nc.vector.tensor_copy(qT[:], ps_q[:])

                ctxT = sm.tile([128, 8, 16], F16, tag="ctxT")
                for half in range(2):
                    ps_e = []
                    for _pi in range(4):
                        ps_e_i = pe4.tile([16, 512], F32, tag="e")
                        ps_e.append(ps_e_i)
                    for hc in range(8):
                        scr = sc.tile([128, 8, 256], F16, tag="scr")
                        for bb in range(8):
                            b = 8 * half + bb
                            nc.vector.tensor_scalar_add(
                                scr[:, bb, :], keys[:, hc, b, :],
                                qT[:, 16 * hc + b:16 * hc + b + 1])
                        nc.scalar.activation(scr[:], scr[:], AF.Tanh)
                        for i in range(4):
                            nc.tensor.matmul(
                                ps_e[i][:], vam[:, hc, :],
                                scr[:].rearrange("p e s -> p (e s)")[
                                    :, 512 * i:512 * i + 512],
                                start=(hc == 0), stop=(hc == 7))
                    e_sb = sm.tile([8, 256], F32, tag="esb")
                    e_flat = sef.tile([128, 512], F32, tag="e512")
                    for i in range(4):
                        nc.vector.tensor_copy(
                            e_flat[32 * i:32 * i + 1, :], ps_e[i][0:1, :])
                    nc.sync.dma_start(
                        e_sb[:],
                        e_flat[0:128:32, :].rearrange("q (b s) -> q b s", b=2))
                    w_sb = sm.tile([8, 256], F32, tag="wsb")
                    sum_e = sm.tile([8, 1], F32, tag="sume")
                    nc.scalar.activation(w_sb[:], e_sb[:], AF.Exp,
                                         accum_out=sum_e[:])
                    rcp = sm.tile([8, 1], F32, tag="rcp")
                    nc.vector.reciprocal(rcp[:], sum_e[:])
                    nc.vector.tensor_scalar_mul(w_sb[:], w_sb[:], rcp[:])
                    nc.sync.dma_start(
                        attn_out[t, 8 * half:8 * half + 8, :], w_sb[:])
                    w16 = sm.tile([8, 256], F16, tag="w16")
                    nc.vector.tensor_copy(w16[:], w_sb[:])
                    wT = sm.tile([128, 2, 8], F16, tag="wT")
                    for sh in range(2):
                        ps_w = pt.tile([128, 8], F16, tag="tp")
                        nc.tensor.transpose(
                            ps_w[:], w16[:, 128 * sh:128 * sh + 128], id8[:])
                        nc.vector.tensor_copy(wT[:, sh, :], ps_w[:])
                    # ctx: per b, per h-chunk: [128s,128h].T @ w[b] col
                    for hc in range(8):
                        ps_c8 = pc.tile([128, 8], F32, tag="ctx")
                        for bb in range(8):
                            b = 8 * half + bb
                            for sh in range(2):
                                nc.tensor.matmul(
                                    ps_c8[:, bb:bb + 1],
                                    big[:, b, sh, 128 * hc:128 * hc + 128],
                                    wT[:, sh, bb:bb + 1],
                                    start=(sh == 0), stop=(sh == 1))
                        nc.vector.tensor_copy(
                            ctxT[:, hc, 8 * half:8 * half + 8], ps_c8[:])
                cin = dram.tile([1024, 16], F16, tag="cin")
                cout = dram.tile([8192, 16], F16, tag="cout")
                nc.sync.dma_start(
                    cin[:].rearrange("(k p) b -> p k b", p=128), ctxT[:])
                nc.gpsimd.collective_compute(
                    "AllGather", ALU.bypass, replica_groups=RG,
                    ins=[cin[:].opt()], outs=[cout[:].opt()])
                ctxg = wc1.tile([128, 8, 8, 16], F16, tag="ctxg")
                cview = cout[:].rearrange("(r k p) b -> k p r b", r=8, p=128)
                for kc in range(8):
                    nc.gpsimd.dma_start(ctxg[:, kc, :, :], cview[kc])

                # ---- gates ----
                ps_g = pg.tile([128, GL], F32, tag="g")
                nc.tensor.matmul(ps_g[:], ones[:], brow[:], start=True,
                                 stop=False)
                for kc in range(8):
                    nc.tensor.matmul(ps_g[:], xs_sb[:, kc, :],
                                     wemb[:, kc, :], start=False, stop=False)
                for kc in range(8):
                    nc.tensor.matmul(ps_g[:], hT_g[pp][:, kc, :],
                                     whh[:, kc, :], start=False, stop=False)
                for kc in range(8):
                    nc.tensor.matmul(
                        ps_g[:],
                        ctxg[:, kc, :, :].rearrange("p r b -> p (r b)"),
                        wctx[:, kc, :], start=False, stop=(kc == 7))
                # ---- pointwise LSTM ----
                nc.scalar.activation(sig_if[:], ps_g[:, 0:256], AF.Tanh,
                                     scale=0.5)
                nc.vector.tensor_scalar(sig_if[:], sig_if[:], 0.5, 0.5,
                                        ALU.mult, ALU.add)
                nc.scalar.activation(tg[:], ps_g[:, 256:384], AF.Tanh)
                nc.scalar.activation(so[:], ps_g[:, 384:512], AF.Tanh,
                                     scale=0.5)
                nc.vector.tensor_scalar(so[:], so[:], 0.5, 0.5,
                                        ALU.mult, ALU.add)
                nc.vector.tensor_tensor(tmp1[:], sig_if[:, 128:256], c_sb[:],
                                        ALU.mult)
                nc.vector.tensor_tensor(tmp2[:], sig_if[:, 0:128], tg[:],
                                        ALU.mult)
                nc.vector.tensor_tensor(c_sb[:], tmp1[:], tmp2[:], ALU.add)
                nc.scalar.activation(tc_f32[:], c_sb[:], AF.Tanh)
                nc.vector.tensor_tensor(h16[:], so[:], tc_f32[:], ALU.mult)

                # ---- h.T slice + AllGather h ----
                ps_h = pt.tile([128, 128], F16, tag="tp")
                nc.tensor.transpose(ps_h[:], h16[:], id128[:])
                hT_own = sm.tile([128, 128], F16, tag="hTown")
                nc.vector.tensor_copy(hT_own[:], ps_h[:])
                hin = dram.tile([128, 128], F16, tag="hin")
                hout = dram.tile([1024, 128], F16, tag="hout")
                nc.sync.dma_start(hin[:], hT_own[:])
                nc.gpsimd.collective_compute(
                    "AllGather", ALU.bypass, replica_groups=RG,
                    ins=[hin[:].opt()], outs=[hout[:].opt()])
                nxt = hT_g[(t + 1) % 2]
                nc.sync.dma_start(
                    nxt[:], hout[:].rearrange("(k p) b -> p k b", p=128))
                # own-b h.T for tail logits (and next step's q input)
                hTb = sm.tile([128, 8, BL], F16, tag="hTb")
                nc.vector.tensor_copy(hTb[:], nxt[:, :, bass.ts(pid, BL)])
                nc.sync.dma_start(hT_store[t], hTb[:])
                hTb_prev = hTb

            # final h (gathered, f16; host casts)
            nc.sync.dma_start(hfin_out[:], hT_g[T % 2][:])

            # ---------- tail: logits + log_softmax ----------
            outw = perm.tile([128, 8, V], F16, tag="wctx")
            outb = perm.tile([1, V], F16, tag="brow")
            nc.sync.dma_start(outw[:], outw_in[:])
            nc.sync.dma_start(outb[:], outb_in[:])
            for tb in range(T // 8):
                lsT = wb.tile([128, 8, 8, BL], F16, tag="xs")
                for tt in range(8):
                    nc.sync.dma_start(lsT[:, :, tt, :], hT_store[8 * tb + tt])
                ps_l = pg.tile([128, V], F32, tag="g")
                nc.tensor.matmul(ps_l[:], ones[:], outb[:], start=True,
                                 stop=False)
                for kc in range(8):
                    nc.tensor.matmul(
                        ps_l[:],
                        lsT[:, kc, :, :].rearrange("p t b -> p (t b)"),
                        outw[:, kc, :], start=False, stop=(kc == 7))
                nmax = sm.tile([128, 1], F32, tag="nmax")
                nc.vector.tensor_reduce(nmax[:], ps_l[:], mybir.AxisListType.X,
                                        ALU.max, negate=True)
                ex = sc.tile([128, V], F32, tag="scr")
                sume = sm.tile([128, 1], F32, tag="sume2")
                nc.scalar.activation(ex[:], ps_l[:], AF.Exp, bias=nmax[:],
                                     accum_out=sume[:])
                lz = sm.tile([128, 1], F32, tag="lz")
                nc.scalar.activation(lz[:], sume[:], getattr(AF, 'Ln', getattr(AF, 'Log', None)))
                nc.vector.tensor_tensor(lz[:], lz[:], nmax[:], ALU.subtract)
                lp = sc.tile([128, V], F32, tag="scr")
                nc.vector.tensor_scalar(lp[:], ps_l[:], lz[:], None,
                                        ALU.subtract)
                nc.sync.dma_start(logp_out[tb], lp[:])
    nc.compile()
    return nc


def _prep_inputs(encoder_outputs, encoder_hidden, encoder_cell, target_tensor,
                 max_length, emb, Wa_w, Wa_b, Ua_w, Ua_b, Va_w, Va_b,
                 W_ih, W_hh, b_ih, b_hh, out_w, out_b, T):
    f16 = np.float16
    tok = np.concatenate([np.zeros((B, 1), target_tensor.dtype),
                          target_tensor[:, :T - 1]], axis=1)  # [B, T]
    xs = emb[tok]                                   # [B, T, H] f32
    in_maps = []
    for j in range(NC):
        bl = slice(BL * j, BL * (j + 1))
        encl = encoder_outputs[bl]                  # [16, 256, 1024]
        gsel = np.concatenate([np.arange(128 * j + g * H,
                                         128 * j + g * H + 128)
                               for g in range(4)])  # own gate dims
        m = {}
        m["enc_sb"] = np.ascontiguousarray(
            encl.reshape(BL, 2, 128, H).transpose(2, 0, 1, 3)).astype(f16)
        m["encT_sb"] = np.ascontiguousarray(
            encl.transpose(2, 0, 1).reshape(8, 128, BL, S)
            .transpose(1, 0, 2, 3)).astype(f16)
        m["UaT"] = np.ascontiguousarray(
            Ua_w.T.reshape(8, 128, 8, 128).transpose(1, 0, 2, 3)).astype(f16)
        m["kbias"] = np.ascontiguousarray(
            (Ua_b + Wa_b).reshape(8, 128).T).astype(np.float32)
        m["WaT"] = np.ascontiguousarray(
            Wa_w.T.reshape(8, 128, 8, 128).transpose(1, 0, 2, 3)).astype(f16)
        m["VaM"] = np.ascontiguousarray(
            np.broadcast_to(Va_w[0].reshape(8, 128).T[:, :, None],
                            (128, 8, 16))).astype(f16)
        m["W_embT"] = np.ascontiguousarray(
            W_ih[gsel, :H].T.reshape(8, 128, GL)
            .transpose(1, 0, 2)).astype(f16)
        m["W_ctxT"] = np.ascontiguousarray(
            W_ih[gsel, H:].T.reshape(8, 128, GL)
            .transpose(1, 0, 2)).astype(f16)
        m["W_hhT"] = np.ascontiguousarray(
            W_hh[gsel].T.reshape(8, 128, GL).transpose(1, 0, 2)).astype(f16)
        m["bias_row"] = (b_ih + b_hh)[gsel][None, :].astype(f16)
        m["ones128"] = np.ones((1, 128), f16)
        m["ident8"] = np.eye(8, dtype=f16)
        m["ident128"] = np.eye(128, dtype=f16)
        m["h0T"] = np.ascontiguousarray(
            encoder_hidden[0].T.reshape(8, 128, 128)
            .transpose(1, 0, 2)).astype(f16)
        m["c0"] = encoder_cell[0][:, 128 * j:128 * j + 128].astype(np.float32)
        m["xsT"] = np.ascontiguousarray(
            xs[:, :T].transpose(1, 2, 0).reshape(T, 8, 128, B)
            .transpose(0, 2, 1, 3)).astype(f16)
        m["out_wT"] = np.ascontiguousarray(
            out_w.T.reshape(8, 128, V).transpose(1, 0, 2)).astype(f16)
        m["outb_row"] = out_b[None, :].astype(f16)
        in_maps.append(m)
    return in_maps


def T_pad(T):
    return T


def kernel(encoder_outputs, encoder_hidden, encoder_cell, target_tensor,
           max_length, emb, Wa_w, Wa_b, Ua_w, Ua_b, Va_w, Va_b,
           W_ih, W_hh, b_ih, b_hh, out_w, out_b, _trace=False, _T=None):
    args = [np.asarray(a) for a in
            (encoder_outputs, encoder_hidden, encoder_cell, target_tensor,
             max_length, emb, Wa_w, Wa_b, Ua_w, Ua_b, Va_w, Va_b,
             W_ih, W_hh, b_ih, b_hh, out_w, out_b)]
    T = int(_T if _T is not None else args[4])
    if "nc" not in _cache or _cache.get("T") != T:
        _cache["nc"] = build(T)
        _cache["T"] = T
    in_maps = _prep_inputs(*args, T=T)
    res = run_bass_kernel_spmd(_cache["nc"], in_maps, list(range(NC)),
                               trace=_trace)
    if _trace:
        _cache["last"] = res
    # assemble outputs
    logp = np.zeros((B, T, V), np.float32)
    attn = np.zeros((B, T, S), np.float32)
    for j in range(NC):
        r = res.results[j]
        lp = r["logp"].reshape(T // 8, 8, BL, V).transpose(2, 0, 1, 3) \
            .reshape(BL, T, V)
        logp[BL * j:BL * (j + 1)] = lp
        attn[BL * j:BL * (j + 1)] = r["attnw"].transpose(1, 0, 2)
    hf = res.results[0]["hfin"]                     # [128 p, 8 kc, 128 b]
    h_fin = hf.transpose(2, 1, 0).reshape(B, H)     # [b, h]
    return logp, h_fin[None].astype(np.float32), attn


if __name__ == "__main__":
    pass
